# revision 15
# baseline (speedup 1.0000x reference)
"""Bass/Trainium2 kernel for nn_Attention_6983616824195 — v2.

Same math & sharding as v1 (8 cores = batch x key-half; per-core partial
softmax numerator+denominator over its 2048-key half, host combines),
restructured for the measured axon-HW cost profile:

  - HW experiments (exp.py/exp2.py/exp3.py) show measured time is
    dominated by the DMA side: ~4us per dma_start when queued, plus a
    strong per-byte cost, while PE/ACT/DVE compute hides almost
    entirely under the dispatch pipeline (a 480-matmul kernel measures
    ~0ns marginal). So v2 minimizes dma_start count (46 -> 6) and
    bytes (20.7 -> 18.1 MB/core): inputs land in SBUF whole
    (everything is resident; no per-block streaming), weights+mask are
    one merged buffer, K+V are one merged buffer, the output is fp16.
    Measured via test.py: 97855 ns vs the v1 baseline's 263037 ns.
    (fp8 anywhere — matmul inputs or storage — fails the 2e-2 gate:
    the output is a near-uniform weighted mean over 2048 keys, ~25x
    smaller than its elements, which amplifies relative error; even
    V-only fp8 storage measures 2.7e-2.)
  - AV accumulates across all 4 key-blocks of a query-block directly in
    PSUM (one group of 32 matmuls + ones-column), removing the SBUF f32
    accumulator and its 48 DVE drain/add ops.

Per-core dataflow (all matmuls contract over the SBUF partition dim):
  kT[d,s]  = sum_c WK[c]^T.KTB[c]      (per 512-key block)
  vext[s,0:128] = (V.WV)*mask[s]; vext[s,128] = mask[s]
  qT[d,q]  = sum_c WQ[c]^T.QTB[c]      (per 512-query block)
  per qb, per sb:  S^T[s,q] = kT^T.qT  (two [128,1024] psum tiles)
                   e = exp(S^T/sqrt(128))  (ACT, fp16 out)
                   AV[q,0:129] += e^T.vext (psum, accum over all sb)
  O[q,0:129] (fp16) <- AV psum after sb=3; numerator cols 0:128,
  denominator col 128. Host: (num0+num1)/(den0+den1) in f32.
"""

import numpy as np

import jax

try:  # persistent compile cache: repeat calls skip the walrus compile
    jax.config.update("jax_compilation_cache_dir", "/tmp/jaxcache")
    jax.config.update("jax_persistent_cache_min_compile_time_secs", 1.0)
    jax.config.update("jax_persistent_cache_min_entry_size_bytes", 0)
except Exception:
    pass

import concourse.bass as bass
import concourse.tile as tile
import concourse.mybir as mybir
from concourse.bass_utils import run_bass_kernel_spmd

B, L, DM = 4, 4096, 1024
DK = DV = 128
N_CORES = 8
LQ = L                 # queries per core (all 4096 of the batch)
LK = L // 2            # keys per core (2048)
P = 128
NDC = DM // P          # dm chunks (8)
NQB = LQ // 512        # q blocks of 512 (8)
NQT_PER_B = 512 // P   # q tiles per block (4)
NST = LK // P          # s tiles per core (16)
NSB = LK // 512        # key blocks per core (4)
JPB = NST // NSB       # s tiles per key block (4)
VW = DV + 1            # v-ext width (129): 128 dv cols + ones column
SCALE = 1.0 / float(np.sqrt(DK))
WCOL = 3 * NDC * DK + NST  # WB cols: WQ | WK | WV | mask

F32 = mybir.dt.float32
F16 = mybir.dt.float16


def _split_multi_waits(nc, max_waits=1):
    """This walrus build encodes at most one sync-wait per instruction;
    move surplus waits onto preceding NoOps on the same engine."""
    for f in nc.m.functions:
        for bb in f.blocks:
            new_insts = []
            for inst in bb.instructions:
                si = inst.sync_info
                if si is not None and si.on_wait and len(si.on_wait) > max_waits:
                    waits = list(si.on_wait)
                    extra, keep = waits[:-max_waits], waits[-max_waits:]
                    for k, w in enumerate(extra):
                        nop = mybir.InstNoOp(name=f"{inst.name}_wsplit{k}")
                        nop.engine = inst.engine
                        nop.sync_info = mybir.SyncInfo(on_wait=[w], on_update=[])
                        new_insts.append(nop)
                    inst.sync_info = mybir.SyncInfo(
                        on_wait=keep, on_update=list(si.on_update)
                    )
                new_insts.append(inst)
            bb.instructions = new_insts


def build_nc(split_waits=True, bufs_e=4, split_queues=False):
    nc = bass.Bass("TRN2", target_bir_lowering=False, debug=False)

    # Host-blocked layouts, partition-major so every DMA is a contiguous
    # identity copy (1 descriptor per partition; see make_in_maps):
    #   QTB[p, qb*4096 + c*512+u] = Q[b, qb*512+u, c*128+p]
    #   KVB[p, sb*4096 + c*512+u] = K[b, h*2048 + sb*512+u, c*128+p]
    #   KVB[p, 16384 + j*1024 + c*128+q] = V[b, h*2048 + j*128+q, c*128+p]
    #   WB[p, c*128+k]        = WQ[c*128+p, k]   (then WK, WV)
    #   WB[p, 3*1024 + j]     = (mask[b, 0, h*2048 + j*128+p] == 1)  (f16)
    qt_d = nc.dram_tensor("QTB", [P, NQB * NDC * 512], F16, kind="ExternalInput").ap()
    kv_d = nc.dram_tensor("KVB", [P, 2 * NSB * NDC * 512], F16, kind="ExternalInput").ap()
    wb_d = nc.dram_tensor("WB", [P, WCOL], F16, kind="ExternalInput").ap()
    # O[p, (qb*4+t)*129+d]: numerator d=0:128, denominator d=128 for the
    # query qb*512 + t*128 + p, fp16
    o_d = nc.dram_tensor("O", [P, NQB * NQT_PER_B * VW], F16, kind="ExternalOutput").ap()

    with tile.TileContext(nc) as tc:
        from contextlib import ExitStack

        with ExitStack() as ctx:
            # ---- SBUF pools ----
            per = ctx.enter_context(tc.tile_pool(name="per", bufs=1))
            epool = ctx.enter_context(tc.tile_pool(name="e", bufs=bufs_e))
            # ---- PSUM pools: 3*2 + 2*1 = 8 banks ----
            ps = ctx.enter_context(tc.tile_pool(name="ps", bufs=3, space="PSUM"))
            pav = ctx.enter_context(tc.tile_pool(name="pav", bufs=2, space="PSUM"))

            # ---- resident inputs: 5 input dma_starts total ----
            wb = per.tile([P, WCOL], F16)
            nc.sync.dma_start(wb[:], wb_d[:])
            WQ0, WK0, WV0, MK0 = 0, NDC * DK, 2 * NDC * DK, 3 * NDC * DK

            kall = per.tile([P, NSB * NDC * 512], F16)   # 32KB/part
            vall = per.tile([P, NSB * JPB * NDC * P], F16)  # 32KB/part
            qall = per.tile([P, NQB * NDC * 512], F16)   # 64KB/part
            alt = nc.scalar if split_queues else nc.sync
            HKV = NSB * NDC * 512
            nc.sync.dma_start(kall[:], kv_d[:, 0:HKV])
            alt.dma_start(vall[:], kv_d[:, HKV : 2 * HKV])
            HQ = NQB // 2 * NDC * 512
            for h in range(2):
                eng = nc.sync if h == 0 else alt
                eng.dma_start(
                    qall[:, h * HQ : (h + 1) * HQ],
                    qt_d[:, h * HQ : (h + 1) * HQ],
                )

            # ---- persistent working state ----
            mkf = per.tile([P, NST], F32)                 # mask upcast f32
            nc.vector.tensor_copy(mkf[:], wb[:, MK0 : MK0 + NST])
            kT = per.tile([P, NST * P], F16)              # [d, s] 4KB/part
            vext = per.tile([P, NST * VW], F16)           # [s-tiles x 129] 4KB
            qT = per.tile([P, LQ], F16)                   # [d, q] 8KB
            of = per.tile([P, NQB * NQT_PER_B * VW], F16)  # output stage 8.1KB

            # ---- projections (PE) ----
            def k_part(sb):
                psk = ps.tile([P, 1024], F32, tag="pss", name=f"psk{sb}")
                for c in range(NDC):
                    nc.tensor.matmul(
                        psk[:, 0:512],
                        wb[:, WK0 + c * DK : WK0 + (c + 1) * DK],
                        kall[:, sb * NDC * 512 + c * 512 : sb * NDC * 512 + (c + 1) * 512],
                        start=(c == 0),
                        stop=(c == NDC - 1),
                    )
                nc.vector.tensor_copy(kT[:, sb * 512 : (sb + 1) * 512], psk[:, 0:512])

            def v_part(sb):
                for u in range(JPB):
                    j = sb * JPB + u
                    psv = pav.tile([P, 2 * VW], F32, tag="av", name=f"psv{j}")
                    base = sb * JPB * NDC * P + u * NDC * P
                    for c in range(NDC):
                        nc.tensor.matmul(
                            psv[:, 0:DV],
                            vall[:, base + c * P : base + (c + 1) * P],
                            wb[:, WV0 + c * DV : WV0 + (c + 1) * DV],
                            start=(c == 0),
                            stop=(c == NDC - 1),
                        )
                    nc.vector.tensor_scalar_mul(
                        vext[:, j * VW : j * VW + DV], psv[:, 0:DV], mkf[:, j : j + 1]
                    )
                    nc.vector.tensor_copy(
                        vext[:, j * VW + DV : j * VW + VW], wb[:, MK0 + j : MK0 + j + 1]
                    )

            def qproj(qb):
                psq = ps.tile([P, 1024], F32, tag="pss", name=f"psq{qb}")
                for c in range(NDC):
                    nc.tensor.matmul(
                        psq[:, 0:512],
                        wb[:, WQ0 + c * DK : WQ0 + (c + 1) * DK],
                        qall[:, qb * NDC * 512 + c * 512 : qb * NDC * 512 + (c + 1) * 512],
                        start=(c == 0),
                        stop=(c == NDC - 1),
                    )
                nc.vector.tensor_copy(qT[:, qb * 512 : (qb + 1) * 512], psq[:, 0:512])

            # ---- attention (per query-block, accumulate over key-blocks) ----
            def scores_exp(sb, qb):
                ets = []
                for u2 in range(JPB // 2):
                    pss = ps.tile([P, 1024], F32, tag="pss", name=f"pss{sb}_{qb}_{u2}")
                    for v2 in range(2):
                        u = u2 * 2 + v2
                        nc.tensor.matmul(
                            pss[:, v2 * 512 : (v2 + 1) * 512],
                            kT[:, (sb * JPB + u) * P : (sb * JPB + u + 1) * P],
                            qT[:, qb * 512 : (qb + 1) * 512],
                            start=True,
                            stop=True,
                        )
                    et = epool.tile([P, 1024], F16, tag="e", name=f"et{sb}_{qb}_{u2}")
                    nc.scalar.activation(
                        et[:], pss[:], mybir.ActivationFunctionType.Exp, scale=SCALE
                    )
                    ets.append(et)
                return ets

            def av_acc(sb, qb, ets, avps):
                # accumulate into the qb's two psum groups; drain after sb==3
                for tp in range(NQT_PER_B // 2):
                    avp = avps[tp]
                    nmm = 2 * JPB
                    for i in range(nmm):
                        half, u = divmod(i, JPB)
                        t = tp * 2 + half
                        et = ets[u // 2]
                        off = (u % 2) * 512
                        nc.tensor.matmul(
                            avp[:, half * VW : (half + 1) * VW],
                            et[:, off + t * P : off + (t + 1) * P],
                            vext[:, (sb * JPB + u) * VW : (sb * JPB + u + 1) * VW],
                            start=(sb == 0 and i == 0),
                            stop=(sb == NSB - 1 and i == nmm - 1),
                            skip_group_check=True,
                        )
                if sb == NSB - 1:
                    for tp in range(NQT_PER_B // 2):
                        g = (qb * NQT_PER_B + tp * 2) * VW
                        nc.vector.tensor_copy(of[:, g : g + 2 * VW], avps[tp][:])

            # ---- schedule ----
            # proj: K first (needed by all scores), V, then Q blocks.
            for sb in range(NSB):
                k_part(sb)
            for sb in range(NSB):
                v_part(sb)
            for qb in range(NQB):
                qproj(qb)

            # qb-outer / sb-inner with a one-unit software pipeline lag:
            # AV of (sb) overlaps ACT-exp of (sb+1).
            pending = None
            for qb in range(NQB):
                avps = [
                    pav.tile([P, 2 * VW], F32, tag="av", name=f"av{qb}_{tp}")
                    for tp in range(NQT_PER_B // 2)
                ]
                for sb in range(NSB):
                    ets = scores_exp(sb, qb)
                    if pending is not None:
                        av_acc(*pending)
                    pending = (sb, qb, ets, avps)
            av_acc(*pending)

            # ---- single contiguous output DMA ----
            nc.sync.dma_start(o_d[:], of[:])

    if split_waits:
        _split_multi_waits(nc)
    return nc


_NC = None


def _get_nc():
    global _NC
    if _NC is None:
        _NC = build_nc()
    return _NC


def _block2(x, rows):
    """x [S, DM] -> partition-major blocked [P, S//rows * NDC * rows]:
    out[p, blk*NDC*rows + c*rows + u] = x[blk*rows + u, c*P + p]"""
    S = x.shape[0]
    nblk = S // rows
    r = x.reshape(nblk, rows, NDC, P)
    return np.ascontiguousarray(r.transpose(3, 0, 2, 1)).reshape(P, nblk * NDC * rows)


def make_in_maps(Q, K, V, mask, WQ, WK, WV):
    f16 = np.float16
    Q = np.asarray(Q, dtype=np.float32)
    K = np.asarray(K, dtype=np.float32)
    V = np.asarray(V, dtype=np.float32)
    mask = np.asarray(mask)

    def wblock(W):
        w = np.asarray(W, dtype=np.float32).astype(f16)
        return np.ascontiguousarray(w.reshape(NDC, P, DK).transpose(1, 0, 2)).reshape(
            P, NDC * DK
        )

    wqb, wkb, wvb = wblock(WQ), wblock(WK), wblock(WV)

    in_maps = []
    for c in range(N_CORES):
        b, h = c // 2, c % 2
        if h == 0:
            qtb_b = _block2(Q[b].astype(f16), 512)  # shared by both halves
        ksl = slice(h * LK, (h + 1) * LK)
        ktb = _block2(K[b, ksl].astype(f16), 512)
        # vtb[p, j*NDC*P + c*P + q] = V[b, h*2048 + j*128 + q, c*128 + p]
        vtb = np.ascontiguousarray(
            V[b, ksl].astype(f16).reshape(NST, P, NDC, P).transpose(3, 0, 2, 1)
        ).reshape(P, NST * NDC * P)
        mkb = np.ascontiguousarray(
            (mask[b, 0, ksl] == 1).astype(f16).reshape(NST, P).T
        )
        wb = np.concatenate([wqb, wkb, wvb, mkb], axis=1)
        kvb = np.concatenate([ktb, vtb], axis=1)
        in_maps.append({"QTB": qtb_b, "KVB": kvb, "WB": wb})
    return in_maps


def _unblock_o(o):
    """O [P, 32*VW] -> [LQ, VW]: query (t*128+p) at [p, t*VW+d]."""
    return (
        o.astype(np.float32)
        .reshape(P, NQB * NQT_PER_B, VW)
        .transpose(1, 0, 2)
        .reshape(LQ, VW)
    )


def assemble(results):
    out = np.empty((B, L, DV), dtype=np.float32)
    for b in range(B):
        a0 = _unblock_o(results[2 * b]["O"])
        a1 = _unblock_o(results[2 * b + 1]["O"])
        num = a0[:, :DV] + a1[:, :DV]
        den = a0[:, DV:] + a1[:, DV:]
        out[b] = num / den
    return out


def kernel(Q, K, V, mask, WQ, WK, WV):
    in_maps = make_in_maps(Q, K, V, mask, WQ, WK, WV)
    try:
        res = run_bass_kernel_spmd(_get_nc(), in_maps, core_ids=list(range(N_CORES)))
    except Exception:
        # transient device faults (e.g. a wedged core from a prior run)
        # usually clear on retry
        import time as _time

        _time.sleep(2.0)
        res = run_bass_kernel_spmd(_get_nc(), in_maps, core_ids=list(range(N_CORES)))
    return assemble(res.results)


# revision 16
# speedup vs baseline: 7.1027x; 7.1027x over previous
"""Bass/Trainium2 kernel for nn_Attention_6983616824195 — v2.

Same math & sharding as v1 (8 cores = batch x key-half; per-core partial
softmax numerator+denominator over its 2048-key half, host combines),
restructured for the measured axon-HW cost profile:

  - HW experiments (exp.py/exp2.py/exp3.py) show measured time is
    dominated by the DMA side: ~4us per dma_start when queued, plus a
    strong per-byte cost, while PE/ACT/DVE compute hides almost
    entirely under the dispatch pipeline (a 480-matmul kernel measures
    ~0ns marginal). So v2 minimizes dma_start count (46 -> 6) and
    bytes (20.7 -> 18.1 MB/core): inputs land in SBUF whole
    (everything is resident; no per-block streaming), weights+mask are
    one merged buffer, K+V are one merged buffer, the output is fp16.
    Measured via test.py: 97855 ns vs the v1 baseline's 263037 ns.
    (fp8 anywhere — matmul inputs or storage — fails the 2e-2 gate:
    the output is a near-uniform weighted mean over 2048 keys, ~25x
    smaller than its elements, which amplifies relative error; even
    V-only fp8 storage measures 2.7e-2.)
  - AV accumulates across all 4 key-blocks of a query-block directly in
    PSUM (one group of 32 matmuls + ones-column), removing the SBUF f32
    accumulator and its 48 DVE drain/add ops.

Per-core dataflow (all matmuls contract over the SBUF partition dim):
  kT[d,s]  = sum_c WK[c]^T.KTB[c]      (per 512-key block)
  vext[s,0:128] = (V.WV)*mask[s]; vext[s,128] = mask[s]
  qT[d,q]  = sum_c WQ[c]^T.QTB[c]      (per 512-query block)
  per qb, per sb:  S^T[s,q] = kT^T.qT  (two [128,1024] psum tiles)
                   e = exp(S^T/sqrt(128))  (ACT, fp16 out)
                   AV[q,0:129] += e^T.vext (psum, accum over all sb)
  O[q,0:129] (fp16) <- AV psum after sb=3; numerator cols 0:128,
  denominator col 128. Host: (num0+num1)/(den0+den1) in f32.
"""

import numpy as np

import jax

try:  # persistent compile cache: repeat calls skip the walrus compile
    jax.config.update("jax_compilation_cache_dir", "/tmp/jaxcache")
    jax.config.update("jax_persistent_cache_min_compile_time_secs", 1.0)
    jax.config.update("jax_persistent_cache_min_entry_size_bytes", 0)
except Exception:
    pass

import concourse.bass as bass
import concourse.tile as tile
import concourse.mybir as mybir
from concourse.bass_utils import run_bass_kernel_spmd

B, L, DM = 4, 4096, 1024
DK = DV = 128
N_CORES = 8
LQ = L                 # queries per core (all 4096 of the batch)
LK = L // 2            # keys per core (2048)
P = 128
NDC = DM // P          # dm chunks (8)
NQB = LQ // 512        # q blocks of 512 (8)
NQT_PER_B = 512 // P   # q tiles per block (4)
NST = LK // P          # s tiles per core (16)
NSB = LK // 512        # key blocks per core (4)
JPB = NST // NSB       # s tiles per key block (4)
VW = DV + 1            # v-ext width (129): 128 dv cols + ones column
SCALE = 1.0 / float(np.sqrt(DK))
WCOL = 3 * NDC * DK + NST  # WB cols: WQ | WK | WV | mask

F32 = mybir.dt.float32
F16 = mybir.dt.float16


def _split_multi_waits(nc, max_waits=1):
    """This walrus build encodes at most one sync-wait per instruction;
    move surplus waits onto preceding NoOps on the same engine."""
    for f in nc.m.functions:
        for bb in f.blocks:
            new_insts = []
            for inst in bb.instructions:
                si = inst.sync_info
                if si is not None and si.on_wait and len(si.on_wait) > max_waits:
                    waits = list(si.on_wait)
                    extra, keep = waits[:-max_waits], waits[-max_waits:]
                    for k, w in enumerate(extra):
                        nop = mybir.InstNoOp(name=f"{inst.name}_wsplit{k}")
                        nop.engine = inst.engine
                        nop.sync_info = mybir.SyncInfo(on_wait=[w], on_update=[])
                        new_insts.append(nop)
                    inst.sync_info = mybir.SyncInfo(
                        on_wait=keep, on_update=list(si.on_update)
                    )
                new_insts.append(inst)
            bb.instructions = new_insts


def build_nc(split_waits=True, bufs_e=4, split_queues=False):
    nc = bass.Bass("TRN2", target_bir_lowering=False, debug=False)

    # Host-blocked layouts (see make_in_maps):
    #   QTB[qb*128+p, c*512+u] = Q[b, qb*512+u, c*128+p]
    #   KVB[sb*128+p, c*512+u]       = K[b, h*2048 + sb*512+u, c*128+p]
    #   KVB[512 + sb*128+p, u*1024 + c*128+q] = V[b, h*2048+(4*sb+u)*128+q, c*128+p]
    #   WB[p, c*128+k]        = WQ[c*128+p, k]   (then WK, WV)
    #   WB[p, 3*1024 + j]     = (mask[b, 0, h*2048 + j*128+p] == 1)  (f16)
    qt_d = nc.dram_tensor("QTB", [NQB * P, NDC * 512], F16, kind="ExternalInput").ap()
    kv_d = nc.dram_tensor("KVB", [2 * NSB * P, NDC * 512], F16, kind="ExternalInput").ap()
    wb_d = nc.dram_tensor("WB", [P, WCOL], F16, kind="ExternalInput").ap()
    # numerator (cols 0:128) + denominator (col 128) per query, fp16
    o_d = nc.dram_tensor("O", [LQ, VW], F16, kind="ExternalOutput").ap()

    with tile.TileContext(nc) as tc:
        from contextlib import ExitStack

        with ExitStack() as ctx:
            # ---- SBUF pools ----
            per = ctx.enter_context(tc.tile_pool(name="per", bufs=1))
            epool = ctx.enter_context(tc.tile_pool(name="e", bufs=bufs_e))
            # ---- PSUM pools: 3*2 + 2*1 = 8 banks ----
            ps = ctx.enter_context(tc.tile_pool(name="ps", bufs=3, space="PSUM"))
            pav = ctx.enter_context(tc.tile_pool(name="pav", bufs=2, space="PSUM"))

            # ---- resident inputs: 5 input dma_starts total ----
            wb = per.tile([P, WCOL], F16)
            nc.sync.dma_start(wb[:], wb_d[:])
            WQ0, WK0, WV0, MK0 = 0, NDC * DK, 2 * NDC * DK, 3 * NDC * DK

            kall = per.tile([P, NSB * NDC * 512], F16)   # 32KB/part
            vall = per.tile([P, NSB * JPB * NDC * P], F16)  # 32KB/part
            qall = per.tile([P, NQB * NDC * 512], F16)   # 64KB/part
            alt = nc.scalar if split_queues else nc.sync
            nc.sync.dma_start(
                kall[:].rearrange("p (n m) -> p n m", n=NSB),
                kv_d[0 : NSB * P].rearrange("(n p) m -> p n m", p=P),
            )
            alt.dma_start(
                vall[:].rearrange("p (n m) -> p n m", n=NSB),
                kv_d[NSB * P : 2 * NSB * P].rearrange("(n p) m -> p n m", p=P),
            )
            for h in range(2):
                eng = nc.sync if h == 0 else alt
                eng.dma_start(
                    qall[:, h * NQB // 2 * NDC * 512 : (h + 1) * NQB // 2 * NDC * 512]
                    .rearrange("p (n m) -> p n m", n=NQB // 2),
                    qt_d[h * NQB // 2 * P : (h + 1) * NQB // 2 * P]
                    .rearrange("(n p) m -> p n m", p=P),
                )

            # ---- persistent working state ----
            mkf = per.tile([P, NST], F32)                 # mask upcast f32
            nc.vector.tensor_copy(mkf[:], wb[:, MK0 : MK0 + NST])
            kT = per.tile([P, NST * P], F16)              # [d, s] 4KB/part
            vext = per.tile([P, NST * VW], F16)           # [s-tiles x 129] 4KB
            qT = per.tile([P, LQ], F16)                   # [d, q] 8KB
            of = per.tile([P, NQB * NQT_PER_B * VW], F16)  # output stage 8.1KB

            # ---- projections (PE) ----
            def k_part(sb):
                psk = ps.tile([P, 1024], F32, tag="pss", name=f"psk{sb}")
                for c in range(NDC):
                    nc.tensor.matmul(
                        psk[:, 0:512],
                        wb[:, WK0 + c * DK : WK0 + (c + 1) * DK],
                        kall[:, sb * NDC * 512 + c * 512 : sb * NDC * 512 + (c + 1) * 512],
                        start=(c == 0),
                        stop=(c == NDC - 1),
                    )
                nc.vector.tensor_copy(kT[:, sb * 512 : (sb + 1) * 512], psk[:, 0:512])

            def v_part(sb):
                for u in range(JPB):
                    j = sb * JPB + u
                    psv = pav.tile([P, 2 * VW], F32, tag="av", name=f"psv{j}")
                    base = sb * JPB * NDC * P + u * NDC * P
                    for c in range(NDC):
                        nc.tensor.matmul(
                            psv[:, 0:DV],
                            vall[:, base + c * P : base + (c + 1) * P],
                            wb[:, WV0 + c * DV : WV0 + (c + 1) * DV],
                            start=(c == 0),
                            stop=(c == NDC - 1),
                        )
                    nc.vector.tensor_scalar_mul(
                        vext[:, j * VW : j * VW + DV], psv[:, 0:DV], mkf[:, j : j + 1]
                    )
                    nc.vector.tensor_copy(
                        vext[:, j * VW + DV : j * VW + VW], wb[:, MK0 + j : MK0 + j + 1]
                    )

            def qproj(qb):
                psq = ps.tile([P, 1024], F32, tag="pss", name=f"psq{qb}")
                for c in range(NDC):
                    nc.tensor.matmul(
                        psq[:, 0:512],
                        wb[:, WQ0 + c * DK : WQ0 + (c + 1) * DK],
                        qall[:, qb * NDC * 512 + c * 512 : qb * NDC * 512 + (c + 1) * 512],
                        start=(c == 0),
                        stop=(c == NDC - 1),
                    )
                nc.vector.tensor_copy(qT[:, qb * 512 : (qb + 1) * 512], psq[:, 0:512])

            # ---- attention (per query-block, accumulate over key-blocks) ----
            def scores_exp(sb, qb):
                ets = []
                for u2 in range(JPB // 2):
                    pss = ps.tile([P, 1024], F32, tag="pss", name=f"pss{sb}_{qb}_{u2}")
                    for v2 in range(2):
                        u = u2 * 2 + v2
                        nc.tensor.matmul(
                            pss[:, v2 * 512 : (v2 + 1) * 512],
                            kT[:, (sb * JPB + u) * P : (sb * JPB + u + 1) * P],
                            qT[:, qb * 512 : (qb + 1) * 512],
                            start=True,
                            stop=True,
                        )
                    et = epool.tile([P, 1024], F16, tag="e", name=f"et{sb}_{qb}_{u2}")
                    nc.scalar.activation(
                        et[:], pss[:], mybir.ActivationFunctionType.Exp, scale=SCALE
                    )
                    ets.append(et)
                return ets

            def av_acc(sb, qb, ets, avps):
                # accumulate into the qb's two psum groups; drain after sb==3
                for tp in range(NQT_PER_B // 2):
                    avp = avps[tp]
                    nmm = 2 * JPB
                    for i in range(nmm):
                        half, u = divmod(i, JPB)
                        t = tp * 2 + half
                        et = ets[u // 2]
                        off = (u % 2) * 512
                        nc.tensor.matmul(
                            avp[:, half * VW : (half + 1) * VW],
                            et[:, off + t * P : off + (t + 1) * P],
                            vext[:, (sb * JPB + u) * VW : (sb * JPB + u + 1) * VW],
                            start=(sb == 0 and i == 0),
                            stop=(sb == NSB - 1 and i == nmm - 1),
                            skip_group_check=True,
                        )
                if sb == NSB - 1:
                    for tp in range(NQT_PER_B // 2):
                        g = (qb * NQT_PER_B + tp * 2) * VW
                        nc.vector.tensor_copy(of[:, g : g + 2 * VW], avps[tp][:])

            # ---- schedule ----
            # proj: K first (needed by all scores), V, then Q blocks.
            for sb in range(NSB):
                k_part(sb)
            for sb in range(NSB):
                v_part(sb)
            for qb in range(NQB):
                qproj(qb)

            # qb-outer / sb-inner with a one-unit software pipeline lag:
            # AV of (sb) overlaps ACT-exp of (sb+1).
            pending = None
            for qb in range(NQB):
                avps = [
                    pav.tile([P, 2 * VW], F32, tag="av", name=f"av{qb}_{tp}")
                    for tp in range(NQT_PER_B // 2)
                ]
                for sb in range(NSB):
                    ets = scores_exp(sb, qb)
                    if pending is not None:
                        av_acc(*pending)
                    pending = (sb, qb, ets, avps)
            av_acc(*pending)

            # ---- single output DMA ----
            nc.sync.dma_start(
                o_d.rearrange("(t p) d -> p t d", p=P),
                of[:].rearrange("p (t d) -> p t d", d=VW),
            )

    if split_waits:
        _split_multi_waits(nc)
    return nc


_NC = None


def _get_nc():
    global _NC
    if _NC is None:
        _NC = build_nc()
    return _NC


def _block2(x, rows):
    """x [S, DM] -> blocked [S//rows * P, NDC*rows]:
    out[blk*P + p, c*rows + u] = x[blk*rows + u, c*P + p]"""
    S = x.shape[0]
    nblk = S // rows
    r = x.reshape(nblk, rows, NDC, P)
    return np.ascontiguousarray(r.transpose(0, 3, 2, 1)).reshape(nblk * P, NDC * rows)


def make_in_maps(Q, K, V, mask, WQ, WK, WV):
    f16 = np.float16
    Q = np.asarray(Q, dtype=np.float32)
    K = np.asarray(K, dtype=np.float32)
    V = np.asarray(V, dtype=np.float32)
    mask = np.asarray(mask)

    def wblock(W):
        w = np.asarray(W, dtype=np.float32).astype(f16)
        return np.ascontiguousarray(w.reshape(NDC, P, DK).transpose(1, 0, 2)).reshape(
            P, NDC * DK
        )

    wqb, wkb, wvb = wblock(WQ), wblock(WK), wblock(WV)

    in_maps = []
    for c in range(N_CORES):
        b, h = c // 2, c % 2
        if h == 0:
            qtb_b = _block2(Q[b].astype(f16), 512)  # shared by both halves
        ksl = slice(h * LK, (h + 1) * LK)
        ktb = _block2(K[b, ksl].astype(f16), 512)
        vtb = _block2(V[b, ksl].astype(f16), P)  # [16*128, 1024]
        vtb = np.ascontiguousarray(
            vtb.reshape(NSB, JPB, P, NDC * P).transpose(0, 2, 1, 3)
        ).reshape(NSB * P, JPB * NDC * P)
        mkb = np.ascontiguousarray(
            (mask[b, 0, ksl] == 1).astype(f16).reshape(NST, P).T
        )
        wb = np.concatenate([wqb, wkb, wvb, mkb], axis=1)
        kvb = np.concatenate([ktb, vtb], axis=0)
        in_maps.append({"QTB": qtb_b, "KVB": kvb, "WB": wb})
    return in_maps


def assemble(results):
    out = np.empty((B, L, DV), dtype=np.float32)
    for b in range(B):
        a0 = results[2 * b]["O"].astype(np.float32)
        a1 = results[2 * b + 1]["O"].astype(np.float32)
        num = a0[:, :DV] + a1[:, :DV]
        den = a0[:, DV:] + a1[:, DV:]
        out[b] = num / den
    return out


def kernel(Q, K, V, mask, WQ, WK, WV):
    in_maps = make_in_maps(Q, K, V, mask, WQ, WK, WV)
    try:
        res = run_bass_kernel_spmd(_get_nc(), in_maps, core_ids=list(range(N_CORES)))
    except Exception:
        # transient device faults (e.g. a wedged core from a prior run)
        # usually clear on retry
        import time as _time

        _time.sleep(2.0)
        res = run_bass_kernel_spmd(_get_nc(), in_maps, core_ids=list(range(N_CORES)))
    return assemble(res.results)
